# revision 1
# baseline (speedup 1.0000x reference)
"""Trainium2 Bass kernel for nn_BatchTrainableButterfly.

The reference applies, per mesh-batch b, a trainable butterfly network
(10 levels of phase shifters + 2x2 directional couplers with butterfly
permutations, plus a final phase layer and bit-reversals) to every token
row x[n, :].  For fixed phases the whole network is a linear map on
C^1024, so out[b] = x @ W_b with W_b = network_b(I_1024) — a 1024x1024
complex64 matrix that is cheap to build on host (O(L^2 log L) total).

Device work per core (8 cores = 4 mesh-batches x 2 token halves):
  out_half[b] = x_half @ W_b as real fp32r matmuls on TensorE:
    re = xr@Wr + xi@(-Wi),  im = xr@Wi + xi@Wr
x arrives token-major, so each 128-token tile is transposed on the PE
(L on partitions) to serve as the matmul stationary operand; results
accumulate in PSUM, are interleaved re/im into SBUF and DMA'd out as
complex64-compatible rows.
"""

import math

import numpy as np

import concourse.tile as tile
from concourse import bacc, bass, mybir
from concourse.bass_utils import run_bass_kernel_spmd
from concourse.masks import make_identity

P = 128          # partitions
L = 1024         # butterfly length
N_TOKENS = 4096
MESH_BATCH = 4
N_CORES = 8
T = (N_TOKENS * MESH_BATCH) // N_CORES  # 2048 token-rows per core
NT = T // P      # 16 token tiles per core
KC = L // P      # 8 contraction chunks
NLEV = int(math.log2(L))  # 10

F32 = mybir.dt.float32
F32R = mybir.dt.float32r
BF16 = mybir.dt.bfloat16

TC = 512          # tokens per pipeline chunk (v3)
NCH = T // TC     # 4 chunks

TRACE = False
LAST_RESULTS = None
VERSION = 3       # 2 = single full-W matmul, 3 = two-stage factorization

# ----------------------------------------------------------------------
# Host side: build the per-batch transfer matrices from the phases.
# ----------------------------------------------------------------------


def _bitrev(n):
    m = int(math.log2(n))
    perm = np.arange(n).reshape(n, 1)
    for _ in range(m):
        n1 = perm.shape[0] // 2
        perm = np.hstack((perm[:n1], perm[n1:]))
    return perm.squeeze(0)


def _forward_indices(length):
    idx = []
    ar = np.arange(length)
    for level in range(int(math.log2(length)) - 1):
        bs = 2 ** (level + 2)
        ind = ar.reshape(-1, length // bs, 2, bs // 2).transpose(0, 1, 3, 2)
        idx.append(ind.reshape(-1))
    return idx


def _build_W(phases):
    """phases (B, NLEV+1, L//2, 2) -> W (B, L, L) complex64 with out = x @ W."""
    B = phases.shape[0]
    br = _bitrev(L)
    fidx = _forward_indices(L)
    dc = np.array([[1.0, 1.0j], [1.0j, 1.0]], dtype=np.complex64)

    x = np.broadcast_to(np.eye(L, dtype=np.complex64), (B, L, L)).copy()
    x = x[..., br]
    for level in range(NLEV):
        x = x.reshape(B, L, L // 2, 2)
        ph = phases[:, level : level + 1, :, :]            # (B, 1, L//2, 2)
        x = x * np.exp(1j * ph.astype(np.complex64))
        x = x @ dc
        x = x.reshape(B, L, L)
        if level < NLEV - 1:
            x = x[..., fidx[level]]
    ph = phases[:, NLEV - 1 : NLEV, :, :].reshape(B, 1, L)
    x = x * np.exp(1j * ph.astype(np.complex64))
    x = x[..., br]
    return (x / np.float32(np.sqrt(L))).astype(np.complex64)


def _rev(v, n):
    r = 0
    for _ in range(n):
        r = (r << 1) | (v & 1)
        v >>= 1
    return r


def _stage_matrices(phases):
    """Two-stage factorization of the butterfly network.

    Stage A = input bitrev + levels 0..6 (perms 0..5, no trailing perm):
    block-diagonal; column-block g is fed by x columns {i : i = 8p + r},
    r = rev3(g).  Stage B = perm fidx[6] + levels 7..9 + final phase +
    final bitrev + scale: per-position 8x8 mixing across the 8 blocks.

    Returns per batch the PE stationaries:
      Astat[b, r] (128,128) cplx : lhsT with K=p (x idx 8p+r), M=pos.
      Bstat[b,t2] (128,128) cplx : lhsT with K = g*16+s (source y(g, t2*16+s)),
                                   M = v*8+m -> out col j = 128m + 8v + rev3(t2).
    Cross-component entries of the extracted B submatrix are exactly 0.
    """
    B_ = phases.shape[0]
    br = _bitrev(L)
    fidx = _forward_indices(L)
    dc = np.array([[1.0, 1.0j], [1.0j, 1.0]], dtype=np.complex64)

    def levels(x, lo, hi, pre_br=False, post_final=False, pre_perm=None):
        if pre_br:
            x = x[..., br]
        if pre_perm is not None:
            x = x[..., pre_perm]
        for level in range(lo, hi):
            x = x.reshape(B_, L, L // 2, 2)
            x = x * np.exp(1j * phases[:, level, None, :, :].astype(np.complex64))
            x = x @ dc
            x = x.reshape(B_, L, L)
            if level < NLEV - 1 and level != 6:
                x = x[..., fidx[level]]
        if post_final:
            x = x * np.exp(
                1j * phases[:, NLEV - 1, None, :, :].reshape(B_, 1, L).astype(np.complex64)
            )
            x = x[..., br]
            x = x / np.float32(np.sqrt(L))
        return x

    eye = np.broadcast_to(np.eye(L, dtype=np.complex64), (B_, L, L)).copy()
    A = levels(eye.copy(), 0, 7, pre_br=True)
    Bm = levels(eye.copy(), 7, NLEV, post_final=True, pre_perm=fidx[6])

    # Stage-A output row order: row' = s*8 + t2 for pos p'' = t2*16 + s, so the
    # inter-stage shuffle is one plain DMA per g: yA_g[:] -> Bin[g:128:8,:,:]
    # (dst partition k = s*8 + g, free = (t2, tok)).
    ar_ = np.arange(P)
    posperm = (ar_ & 7) * 16 + (ar_ >> 3)          # row' -> p''
    Astat = np.empty((B_, 8, P, P), dtype=np.complex64)
    for r in range(8):
        g = _rev(r, 3)
        Astat[:, r] = A[:, ar_ * 8 + r][:, :, g * P + posperm]

    s_, g_ = np.divmod(ar_, 8)                     # k = s*8 + g
    v_, m_ = np.divmod(ar_, 8)
    Bstat = np.empty((B_, 8, P, P), dtype=np.complex64)
    for t2 in range(8):
        rows = g_ * P + t2 * 16 + s_
        cols = P * m_ + 8 * v_ + _rev(t2, 3)
        Bstat[:, t2] = Bm[:, rows][:, :, cols]
    return Astat, Bstat


# ----------------------------------------------------------------------
# Device side: complex matmul kernel (SPMD, one (batch, half) per core).
# ----------------------------------------------------------------------

_CACHED_NC = None


def _build_program():
    nc = bacc.Bacc(
        "TRN2", target_bir_lowering=False, debug=False, num_devices=N_CORES
    )

    xr_d = nc.declare_dram_parameter("xr", [T, L], F32, isOutput=False)
    xi_d = nc.declare_dram_parameter("xi", [T, L], F32, isOutput=False)
    wr_d = nc.declare_dram_parameter("wr", [L, L], F32R, isOutput=False)
    wi_d = nc.declare_dram_parameter("wi", [L, L], F32R, isOutput=False)
    out_d = nc.declare_dram_parameter("out", [T, 2 * L], F32, isOutput=True)

    with tile.TileContext(nc) as tc:
        with (
            tc.tile_pool(name="const", bufs=1) as const_pool,
            tc.tile_pool(name="w", bufs=1) as w_pool,
            tc.tile_pool(name="x", bufs=3) as x_pool,
            tc.tile_pool(name="xt", bufs=2) as xt_pool,
            tc.tile_pool(name="osb", bufs=3) as o_pool,
            tc.tile_pool(name="ps", bufs=8, space=bass.MemorySpace.PSUM) as ps_pool,
        ):
            ident = const_pool.tile([P, P], F32)
            make_identity(nc, ident[:])

            # Warm the PE HAM while W streams in: dummy transposes keep the
            # tensor engine busy >3.4us so it reaches full clock before the
            # real matmuls start.
            warm = ps_pool.tile([P, 4 * P], F32, tag="ps")
            for _ in range(12):
                for j in range(4):
                    nc.tensor.transpose(
                        warm[:, j * P : (j + 1) * P], ident[:], ident[:]
                    )

            # Stream W into SBUF once: per k-chunk tiles (P x L), natural layout
            # (partition = contraction row within chunk, free = output column).
            # k-major order so the first token tile's accumulation can start
            # after only a few chunks have landed.
            w_sb = {}
            for k in range(KC):
                for nm, dram in (("wr", wr_d), ("wi", wi_d)):
                    t_ = w_pool.tile([P, L], F32R, tag=f"{nm}{k}")
                    nc.sync.dma_start(out=t_[:], in_=dram[k * P : (k + 1) * P, :])
                    w_sb[nm, k] = t_
                # -Wi derived on device: saves a third of the W stream, which
                # gates the kernel head while PE waits on weights.
                nwi = w_pool.tile([P, L], F32R, tag=f"nwi{k}")
                nc.vector.tensor_scalar_mul(nwi[:], w_sb["wi", k][:], -1.0)
                w_sb["nwi", k] = nwi

            for t in range(NT):
                rows = slice(t * P, (t + 1) * P)
                xr_rows = x_pool.tile([P, L], F32, tag="xr_rows")
                xi_rows = x_pool.tile([P, L], F32, tag="xi_rows")
                nc.sync.dma_start(out=xr_rows[:], in_=xr_d[rows, :])
                nc.sync.dma_start(out=xi_rows[:], in_=xi_d[rows, :])

                # Transpose the token tile: xT chunks live at
                # xT[:, k*P:(k+1)*P] = x_rows[:, k*P:(k+1)*P].T
                xrT = xt_pool.tile([P, L], F32R, tag="xrT")
                xiT = xt_pool.tile([P, L], F32R, tag="xiT")
                for src, dst in ((xr_rows, xrT), (xi_rows, xiT)):
                    for g in range(2):
                        tp = ps_pool.tile([P, 4 * P], F32, tag="ps")
                        for j in range(4):
                            k = g * 4 + j
                            nc.tensor.transpose(
                                tp[:, j * P : (j + 1) * P],
                                src[:, k * P : (k + 1) * P],
                                ident[:],
                            )
                        nc.scalar.copy(dst[:, g * 4 * P : (g + 1) * 4 * P], tp[:])

                # Accumulate the four real matmul outputs.
                #   re_n = sum_k xrT_k @ wr_k[n] + xiT_k @ nwi_k[n]
                #   im_n = sum_k xrT_k @ wi_k[n] + xiT_k @ wr_k[n]
                out_sb = o_pool.tile([P, L, 2], F32, tag="out_sb")
                for n in range(2):
                    ncol = slice(n * 512, (n + 1) * 512)
                    acc_re = ps_pool.tile([P, 512], F32, tag="ps")
                    acc_im = ps_pool.tile([P, 512], F32, tag="ps")
                    for k in range(KC):
                        xrT_k = xrT[:, k * P : (k + 1) * P]
                        xiT_k = xiT[:, k * P : (k + 1) * P]
                        first = k == 0
                        last = k == KC - 1
                        nc.tensor.matmul(
                            acc_re[:], xrT_k, w_sb["wr", k][:, ncol],
                            start=first, stop=False,
                        )
                        nc.tensor.matmul(
                            acc_re[:], xiT_k, w_sb["nwi", k][:, ncol],
                            start=False, stop=last,
                        )
                        nc.tensor.matmul(
                            acc_im[:], xrT_k, w_sb["wi", k][:, ncol],
                            start=first, stop=False,
                        )
                        nc.tensor.matmul(
                            acc_im[:], xiT_k, w_sb["wr", k][:, ncol],
                            start=False, stop=last,
                        )
                    # Interleave re/im into complex64 memory order.
                    nc.vector.tensor_copy(out_sb[:, n * 512 : (n + 1) * 512, 0], acc_re[:])
                    nc.vector.tensor_copy(out_sb[:, n * 512 : (n + 1) * 512, 1], acc_im[:])

                nc.sync.dma_start(out=out_d[rows, :], in_=out_sb[:])

    nc.compile()
    return nc


def _build_program_v3():
    # detect_race_conditions=False: the rust race detector false-positives on
    # the stepped-partition shuffle DMA vs writes to a *different* bin buffer
    # (disjoint SBUF regions sharing a shadow zone). Same-tensor deps are
    # tracked normally and validated by the CoreSim numeric check.
    nc = bacc.Bacc(
        "TRN2", target_bir_lowering=False, debug=False, num_devices=N_CORES,
        detect_race_conditions=False,
    )

    xr_d = nc.declare_dram_parameter("xr", [T, L], F32R, isOutput=False)
    xi_d = nc.declare_dram_parameter("xi", [T, L], F32R, isOutput=False)
    ar_d = nc.declare_dram_parameter("ar", [8 * P, P], F32R, isOutput=False)
    ai_d = nc.declare_dram_parameter("ai", [8 * P, P], F32R, isOutput=False)
    nai_d = nc.declare_dram_parameter("nai", [8 * P, P], F32R, isOutput=False)
    br_d = nc.declare_dram_parameter("br", [8 * P, P], BF16, isOutput=False)
    bi_d = nc.declare_dram_parameter("bi", [8 * P, P], BF16, isOutput=False)
    nbi_d = nc.declare_dram_parameter("nbi", [8 * P, P], BF16, isOutput=False)
    out_d = nc.declare_dram_parameter("out", [T, 2 * L], F32, isOutput=True)

    with tile.TileContext(nc) as tc:
        with (
            tc.tile_pool(name="const", bufs=1) as const_pool,
            tc.tile_pool(name="mats", bufs=1) as mat_pool,
            tc.tile_pool(name="x", bufs=8) as x_pool,
            tc.tile_pool(name="xt", bufs=20) as xt_pool,
            tc.tile_pool(name="ya", bufs=12) as ya_pool,
            tc.tile_pool(name="bin", bufs=1) as bin_pool,
            tc.tile_pool(name="yb", bufs=4) as yb_pool,
            tc.tile_pool(name="osb", bufs=4) as o_pool,
            tc.tile_pool(name="ps", bufs=8, space=bass.MemorySpace.PSUM) as ps_pool,
        ):
            ident = const_pool.tile([P, P], F32)
            make_identity(nc, ident[:])
            ident_h = const_pool.tile([P, P], BF16)
            nc.vector.tensor_copy(ident_h[:], ident[:])
            ident_r = const_pool.tile([P, P], F32R)
            nc.vector.tensor_copy(ident_r[:], ident[:])

            # HAM warmup while the (small) stationaries stream in.
            warm = ps_pool.tile([P, 4 * P], F32, tag="ps")
            for _ in range(22):
                for j in range(4):
                    nc.tensor.transpose(
                        warm[:, j * P : (j + 1) * P], ident[:], ident[:]
                    )

            # Persistent double-buffered shuffle destination; memset once so
            # downstream readers of the stepped-partition DMA writes are
            # observable (sim init tracking) — overlaps with warmup/mats DMA.
            bn_bufs = []
            bn_memsets = []
            for i in range(2):
                bnb = bin_pool.tile([P, 8, 2 * TC], BF16, tag=f"bin{i}")
                bn_memsets.append(nc.gpsimd.memset(bnb[:], 0.0))
                bn_bufs.append(bnb)

            # Mats go through the gpsimd SWDGE queues so the 48 dma_starts do
            # not serialize ahead of chunk-0 row loads on the two HWDGE queues.
            mats = {}
            for nm, dram, dt_ in (
                ("ar", ar_d, F32R), ("ai", ai_d, F32R), ("nai", nai_d, F32R),
                ("br", br_d, BF16), ("bi", bi_d, BF16), ("nbi", nbi_d, BF16),
            ):
                for r in range(8):
                    t_ = mat_pool.tile([P, P], dt_, tag=f"{nm}{r}")
                    nc.gpsimd.dma_start(out=t_[:], in_=dram[r * P : (r + 1) * P, :])
                    mats[nm, r] = t_

            def emit_front(ch):
                """T_in + stage A + shuffle for chunk ch."""
                tok0 = ch * TC
                rows = {}
                for pl, dram in ((0, xr_d), (1, xi_d)):
                    for tt in range(TC // P):
                        rt = x_pool.tile([P, P, 8], F32R, tag="rows")
                        r0 = tok0 + tt * P
                        eng = nc.scalar if (tt % 2) else nc.sync
                        eng.dma_start(out=rt[:], in_=dram[r0 : r0 + P, :])
                        rows[pl, tt] = rt

                xT = {}
                for pl in range(2):
                    for r in range(8):
                        tp = ps_pool.tile([P, 4 * P], F32R, tag="ps")
                        for tt in range(TC // P):
                            nc.tensor.transpose(
                                tp[:, tt * P : (tt + 1) * P],
                                rows[pl, tt][:, :, r],
                                ident_r[:],
                            )
                        dst = xt_pool.tile([P, TC], F32R, tag="xT")
                        nc.scalar.copy(dst[:], tp[:])
                        xT[pl, r] = dst

                yA = {}
                for r in range(8):
                    g = _rev(r, 3)
                    acr = ps_pool.tile([P, TC], F32, tag="ps")
                    aci = ps_pool.tile([P, TC], F32, tag="ps")
                    nc.tensor.matmul(acr[:], mats["ar", r][:], xT[0, r][:], start=True, stop=False)
                    nc.tensor.matmul(acr[:], mats["nai", r][:], xT[1, r][:], start=False, stop=True)
                    nc.tensor.matmul(aci[:], mats["ai", r][:], xT[0, r][:], start=True, stop=False)
                    nc.tensor.matmul(aci[:], mats["ar", r][:], xT[1, r][:], start=False, stop=True)
                    ya = ya_pool.tile([P, 2 * TC], BF16, tag="ya")
                    nc.vector.tensor_copy(ya[:, 0:TC], acr[:])
                    nc.vector.tensor_copy(ya[:, TC : 2 * TC], aci[:])
                    yA[g] = ya

                # shuffle: Bin[s*8+g, t2, :] = yA[g][s*8+t2, :] — one plain DMA
                # per g; one partition per SBUF port group on both sides.
                bn = bn_bufs[ch % 2]
                for g in range(8):
                    eng = nc.scalar if (g % 2) else nc.sync
                    eng.dma_start(out=bn[g:P:8, :, :], in_=yA[g][:])
                return bn

            def emit_back(ch, bn):
                """Stage B + T_out + interleave + store for chunk ch."""
                tok0 = ch * TC
                out_sb = []
                for tt in range(TC // P):
                    osb = o_pool.tile([P, 2 * L], F32, tag="osb")
                    out_sb.append(osb)
                for t2 in range(8):
                    obr = ps_pool.tile([P, TC], F32, tag="ps")
                    obi = ps_pool.tile([P, TC], F32, tag="ps")
                    b_re = bn[:, t2, 0:TC]
                    b_im = bn[:, t2, TC : 2 * TC]
                    nc.tensor.matmul(obr[:], mats["br", t2][:], b_re, start=True, stop=False)
                    nc.tensor.matmul(obr[:], mats["nbi", t2][:], b_im, start=False, stop=True)
                    nc.tensor.matmul(obi[:], mats["bi", t2][:], b_re, start=True, stop=False)
                    nc.tensor.matmul(obi[:], mats["br", t2][:], b_im, start=False, stop=True)
                    yb = yb_pool.tile([P, 2 * TC], BF16, tag="yb")
                    nc.scalar.copy(yb[:, 0:TC], obr[:])
                    nc.scalar.copy(yb[:, TC:], obi[:])

                    base = 2 * _rev(t2, 3)
                    for tt in range(TC // P):
                        tp2 = ps_pool.tile([P, 2, 16, 8], BF16, tag="ps")
                        nc.tensor.transpose(
                            tp2[:, 0], yb[:, tt * P : (tt + 1) * P], ident_h[:]
                        )
                        nc.tensor.transpose(
                            tp2[:, 1], yb[:, TC + tt * P : TC + (tt + 1) * P], ident_h[:]
                        )
                        osr = out_sb[tt][:].rearrange(
                            "q (m v lo) -> q lo v m", m=8, v=16, lo=16
                        )
                        nc.vector.tensor_copy(osr[:, base : base + 2, :, :], tp2[:])

                for tt in range(TC // P):
                    r0 = tok0 + tt * P
                    eng = nc.scalar if (tt % 2) else nc.sync
                    eng.dma_start(out=out_d[r0 : r0 + P, :], in_=out_sb[tt][:])

            # Software pipeline: back-half of chunk ch-1 is emitted after the
            # front-half (and shuffle issue) of chunk ch, so the PE stream has
            # B/T_out work in hand while chunk ch's shuffle is in flight.
            prev = None
            for ch in range(NCH):
                bn = emit_front(ch)
                if prev is not None:
                    emit_back(prev[0], prev[1])
                prev = (ch, bn)
            emit_back(prev[0], prev[1])

    nc.compile()
    return nc


_CACHED = {}


def kernel(x_re: np.ndarray, x_im: np.ndarray, phases: np.ndarray) -> np.ndarray:
    global LAST_RESULTS

    x_re = np.ascontiguousarray(x_re, dtype=np.float32)
    x_im = np.ascontiguousarray(x_im, dtype=np.float32)
    phases = np.ascontiguousarray(phases, dtype=np.float32)

    half = N_TOKENS // 2
    in_maps = []
    if VERSION == 2:
        W = _build_W(phases)                  # (B, L, L) complex64
        Wr = np.ascontiguousarray(W.real, dtype=np.float32)
        Wi = np.ascontiguousarray(W.imag, dtype=np.float32)
        if 2 not in _CACHED:
            _CACHED[2] = _build_program()
        nc = _CACHED[2]
        for c in range(N_CORES):
            b, h = c // 2, c % 2
            in_maps.append(
                {
                    "xr": x_re[h * half : (h + 1) * half],
                    "xi": x_im[h * half : (h + 1) * half],
                    "wr": Wr[b],
                    "wi": Wi[b],
                }
            )
    else:
        import ml_dtypes

        Astat, Bstat = _stage_matrices(phases)
        ar = np.ascontiguousarray(Astat.real.reshape(MESH_BATCH, 8 * P, P))
        ai = np.ascontiguousarray(Astat.imag.reshape(MESH_BATCH, 8 * P, P))
        br = Bstat.real.reshape(MESH_BATCH, 8 * P, P).astype(ml_dtypes.bfloat16)
        bi = Bstat.imag.reshape(MESH_BATCH, 8 * P, P).astype(ml_dtypes.bfloat16)
        if 3 not in _CACHED:
            _CACHED[3] = _build_program_v3()
        nc = _CACHED[3]
        for c in range(N_CORES):
            b, h = c // 2, c % 2
            in_maps.append(
                {
                    "xr": x_re[h * half : (h + 1) * half],
                    "xi": x_im[h * half : (h + 1) * half],
                    "ar": ar[b],
                    "ai": ai[b],
                    "nai": np.ascontiguousarray(-ai[b]),
                    "br": br[b],
                    "bi": bi[b],
                    "nbi": np.ascontiguousarray(-bi[b]),
                }
            )

    res = run_bass_kernel_spmd(nc, in_maps, list(range(N_CORES)), trace=TRACE)
    LAST_RESULTS = res

    out = np.empty((MESH_BATCH, N_TOKENS, L), dtype=np.complex64)
    for c in range(N_CORES):
        b, h = c // 2, c % 2
        out[b, h * half : (h + 1) * half] = (
            res.results[c]["out"].view(np.complex64).reshape(half, L)
        )
    return out



# revision 6
# speedup vs baseline: 1.3939x; 1.3939x over previous
"""Trainium2 Bass kernel for nn_BatchTrainableButterfly.

The reference applies, per mesh-batch b, a trainable butterfly network
(10 levels of phase shifters + 2x2 directional couplers with butterfly
permutations, plus a final phase layer and bit-reversals) to every token
row x[n, :].  For fixed phases the whole network is a linear map on
C^1024, so out[b] = x @ W_b with W_b = network_b(I_1024) — a 1024x1024
complex64 matrix that is cheap to build on host (O(L^2 log L) total).

Device work per core (8 cores = 4 mesh-batches x 2 token halves):
  out_half[b] = x_half @ W_b as real fp32r matmuls on TensorE:
    re = xr@Wr + xi@(-Wi),  im = xr@Wi + xi@Wr
x arrives token-major, so each 128-token tile is transposed on the PE
(L on partitions) to serve as the matmul stationary operand; results
accumulate in PSUM, are interleaved re/im into SBUF and DMA'd out as
complex64-compatible rows.
"""

import math

import numpy as np

import concourse.tile as tile
from concourse import bacc, bass, mybir
from concourse.bass_utils import run_bass_kernel_spmd
from concourse.masks import make_identity

P = 128          # partitions
L = 1024         # butterfly length
N_TOKENS = 4096
MESH_BATCH = 4
N_CORES = 8
T = (N_TOKENS * MESH_BATCH) // N_CORES  # 2048 token-rows per core
NT = T // P      # 16 token tiles per core
KC = L // P      # 8 contraction chunks
NLEV = int(math.log2(L))  # 10

F32 = mybir.dt.float32
F32R = mybir.dt.float32r
BF16 = mybir.dt.bfloat16

TC = 512          # tokens per pipeline chunk (v3)
NCH = T // TC     # 4 chunks

TRACE = False
LAST_RESULTS = None
VERSION = 4       # 2 = full-W matmul, 3 = two-stage, 4 = host-transposed two-stage

# ----------------------------------------------------------------------
# Host side: build the per-batch transfer matrices from the phases.
# ----------------------------------------------------------------------


def _bitrev(n):
    m = int(math.log2(n))
    perm = np.arange(n).reshape(n, 1)
    for _ in range(m):
        n1 = perm.shape[0] // 2
        perm = np.hstack((perm[:n1], perm[n1:]))
    return perm.squeeze(0)


def _forward_indices(length):
    idx = []
    ar = np.arange(length)
    for level in range(int(math.log2(length)) - 1):
        bs = 2 ** (level + 2)
        ind = ar.reshape(-1, length // bs, 2, bs // 2).transpose(0, 1, 3, 2)
        idx.append(ind.reshape(-1))
    return idx


def _build_W(phases):
    """phases (B, NLEV+1, L//2, 2) -> W (B, L, L) complex64 with out = x @ W."""
    B = phases.shape[0]
    br = _bitrev(L)
    fidx = _forward_indices(L)
    dc = np.array([[1.0, 1.0j], [1.0j, 1.0]], dtype=np.complex64)

    x = np.broadcast_to(np.eye(L, dtype=np.complex64), (B, L, L)).copy()
    x = x[..., br]
    for level in range(NLEV):
        x = x.reshape(B, L, L // 2, 2)
        ph = phases[:, level : level + 1, :, :]            # (B, 1, L//2, 2)
        x = x * np.exp(1j * ph.astype(np.complex64))
        x = x @ dc
        x = x.reshape(B, L, L)
        if level < NLEV - 1:
            x = x[..., fidx[level]]
    ph = phases[:, NLEV - 1 : NLEV, :, :].reshape(B, 1, L)
    x = x * np.exp(1j * ph.astype(np.complex64))
    x = x[..., br]
    return (x / np.float32(np.sqrt(L))).astype(np.complex64)


def _rev(v, n):
    r = 0
    for _ in range(n):
        r = (r << 1) | (v & 1)
        v >>= 1
    return r


def _stage_matrices(phases):
    """Two-stage factorization of the butterfly network.

    Stage A = input bitrev + levels 0..6 (perms 0..5, no trailing perm):
    block-diagonal; column-block g is fed by x columns {i : i = 8p + r},
    r = rev3(g).  Stage B = perm fidx[6] + levels 7..9 + final phase +
    final bitrev + scale: per-position 8x8 mixing across the 8 blocks.

    Returns per batch the PE stationaries:
      Astat[b, r] (128,128) cplx : lhsT with K=p (x idx 8p+r), M=pos.
      Bstat[b,t2] (128,128) cplx : lhsT with K = g*16+s (source y(g, t2*16+s)),
                                   M = v*8+m -> out col j = 128m + 8v + rev3(t2).
    Cross-component entries of the extracted B submatrix are exactly 0.
    """
    B_ = phases.shape[0]
    br = _bitrev(L)
    fidx = _forward_indices(L)
    dc = np.array([[1.0, 1.0j], [1.0j, 1.0]], dtype=np.complex64)

    def levels(x, lo, hi, pre_br=False, post_final=False, pre_perm=None):
        if pre_br:
            x = x[..., br]
        if pre_perm is not None:
            x = x[..., pre_perm]
        for level in range(lo, hi):
            x = x.reshape(B_, L, L // 2, 2)
            x = x * np.exp(1j * phases[:, level, None, :, :].astype(np.complex64))
            x = x @ dc
            x = x.reshape(B_, L, L)
            if level < NLEV - 1 and level != 6:
                x = x[..., fidx[level]]
        if post_final:
            x = x * np.exp(
                1j * phases[:, NLEV - 1, None, :, :].reshape(B_, 1, L).astype(np.complex64)
            )
            x = x[..., br]
            x = x / np.float32(np.sqrt(L))
        return x

    eye = np.broadcast_to(np.eye(L, dtype=np.complex64), (B_, L, L)).copy()
    A = levels(eye.copy(), 0, 7, pre_br=True)
    Bm = levels(eye.copy(), 7, NLEV, post_final=True, pre_perm=fidx[6])

    # Stage-A output row order: row' = s*8 + t2 for pos p'' = t2*16 + s, so the
    # inter-stage shuffle is one plain DMA per g: yA_g[:] -> Bin[g:128:8,:,:]
    # (dst partition k = s*8 + g, free = (t2, tok)).
    ar_ = np.arange(P)
    posperm = (ar_ & 7) * 16 + (ar_ >> 3)          # row' -> p''
    Astat = np.empty((B_, 8, P, P), dtype=np.complex64)
    for r in range(8):
        g = _rev(r, 3)
        Astat[:, r] = A[:, ar_ * 8 + r][:, :, g * P + posperm]

    s_, g_ = np.divmod(ar_, 8)                     # k = s*8 + g
    v_, m_ = np.divmod(ar_, 8)
    Bstat = np.empty((B_, 8, P, P), dtype=np.complex64)
    for t2 in range(8):
        rows = g_ * P + t2 * 16 + s_
        cols = P * m_ + 8 * v_ + _rev(t2, 3)
        Bstat[:, t2] = Bm[:, rows][:, :, cols]
    return Astat, Bstat


# ----------------------------------------------------------------------
# Device side: complex matmul kernel (SPMD, one (batch, half) per core).
# ----------------------------------------------------------------------

_CACHED_NC = None


def _build_program():
    nc = bacc.Bacc(
        "TRN2", target_bir_lowering=False, debug=False, num_devices=N_CORES
    )

    xr_d = nc.declare_dram_parameter("xr", [T, L], F32, isOutput=False)
    xi_d = nc.declare_dram_parameter("xi", [T, L], F32, isOutput=False)
    wr_d = nc.declare_dram_parameter("wr", [L, L], F32R, isOutput=False)
    wi_d = nc.declare_dram_parameter("wi", [L, L], F32R, isOutput=False)
    out_d = nc.declare_dram_parameter("out", [T, 2 * L], F32, isOutput=True)

    with tile.TileContext(nc) as tc:
        with (
            tc.tile_pool(name="const", bufs=1) as const_pool,
            tc.tile_pool(name="w", bufs=1) as w_pool,
            tc.tile_pool(name="x", bufs=3) as x_pool,
            tc.tile_pool(name="xt", bufs=2) as xt_pool,
            tc.tile_pool(name="osb", bufs=3) as o_pool,
            tc.tile_pool(name="ps", bufs=8, space=bass.MemorySpace.PSUM) as ps_pool,
        ):
            ident = const_pool.tile([P, P], F32)
            make_identity(nc, ident[:])

            # Warm the PE HAM while W streams in: dummy transposes keep the
            # tensor engine busy >3.4us so it reaches full clock before the
            # real matmuls start.
            warm = ps_pool.tile([P, 4 * P], F32, tag="ps")
            for _ in range(12):
                for j in range(4):
                    nc.tensor.transpose(
                        warm[:, j * P : (j + 1) * P], ident[:], ident[:]
                    )

            # Stream W into SBUF once: per k-chunk tiles (P x L), natural layout
            # (partition = contraction row within chunk, free = output column).
            # k-major order so the first token tile's accumulation can start
            # after only a few chunks have landed.
            w_sb = {}
            for k in range(KC):
                for nm, dram in (("wr", wr_d), ("wi", wi_d)):
                    t_ = w_pool.tile([P, L], F32R, tag=f"{nm}{k}")
                    nc.sync.dma_start(out=t_[:], in_=dram[k * P : (k + 1) * P, :])
                    w_sb[nm, k] = t_
                # -Wi derived on device: saves a third of the W stream, which
                # gates the kernel head while PE waits on weights.
                nwi = w_pool.tile([P, L], F32R, tag=f"nwi{k}")
                nc.vector.tensor_scalar_mul(nwi[:], w_sb["wi", k][:], -1.0)
                w_sb["nwi", k] = nwi

            for t in range(NT):
                rows = slice(t * P, (t + 1) * P)
                xr_rows = x_pool.tile([P, L], F32, tag="xr_rows")
                xi_rows = x_pool.tile([P, L], F32, tag="xi_rows")
                nc.sync.dma_start(out=xr_rows[:], in_=xr_d[rows, :])
                nc.sync.dma_start(out=xi_rows[:], in_=xi_d[rows, :])

                # Transpose the token tile: xT chunks live at
                # xT[:, k*P:(k+1)*P] = x_rows[:, k*P:(k+1)*P].T
                xrT = xt_pool.tile([P, L], F32R, tag="xrT")
                xiT = xt_pool.tile([P, L], F32R, tag="xiT")
                for src, dst in ((xr_rows, xrT), (xi_rows, xiT)):
                    for g in range(2):
                        tp = ps_pool.tile([P, 4 * P], F32, tag="ps")
                        for j in range(4):
                            k = g * 4 + j
                            nc.tensor.transpose(
                                tp[:, j * P : (j + 1) * P],
                                src[:, k * P : (k + 1) * P],
                                ident[:],
                            )
                        nc.scalar.copy(dst[:, g * 4 * P : (g + 1) * 4 * P], tp[:])

                # Accumulate the four real matmul outputs.
                #   re_n = sum_k xrT_k @ wr_k[n] + xiT_k @ nwi_k[n]
                #   im_n = sum_k xrT_k @ wi_k[n] + xiT_k @ wr_k[n]
                out_sb = o_pool.tile([P, L, 2], F32, tag="out_sb")
                for n in range(2):
                    ncol = slice(n * 512, (n + 1) * 512)
                    acc_re = ps_pool.tile([P, 512], F32, tag="ps")
                    acc_im = ps_pool.tile([P, 512], F32, tag="ps")
                    for k in range(KC):
                        xrT_k = xrT[:, k * P : (k + 1) * P]
                        xiT_k = xiT[:, k * P : (k + 1) * P]
                        first = k == 0
                        last = k == KC - 1
                        nc.tensor.matmul(
                            acc_re[:], xrT_k, w_sb["wr", k][:, ncol],
                            start=first, stop=False,
                        )
                        nc.tensor.matmul(
                            acc_re[:], xiT_k, w_sb["nwi", k][:, ncol],
                            start=False, stop=last,
                        )
                        nc.tensor.matmul(
                            acc_im[:], xrT_k, w_sb["wi", k][:, ncol],
                            start=first, stop=False,
                        )
                        nc.tensor.matmul(
                            acc_im[:], xiT_k, w_sb["wr", k][:, ncol],
                            start=False, stop=last,
                        )
                    # Interleave re/im into complex64 memory order.
                    nc.vector.tensor_copy(out_sb[:, n * 512 : (n + 1) * 512, 0], acc_re[:])
                    nc.vector.tensor_copy(out_sb[:, n * 512 : (n + 1) * 512, 1], acc_im[:])

                nc.sync.dma_start(out=out_d[rows, :], in_=out_sb[:])

    nc.compile()
    return nc


def _build_program_v3():
    # detect_race_conditions=False: the rust race detector false-positives on
    # the stepped-partition shuffle DMA vs writes to a *different* bin buffer
    # (disjoint SBUF regions sharing a shadow zone). Same-tensor deps are
    # tracked normally and validated by the CoreSim numeric check.
    nc = bacc.Bacc(
        "TRN2", target_bir_lowering=False, debug=False, num_devices=N_CORES,
        detect_race_conditions=False,
    )

    xr_d = nc.declare_dram_parameter("xr", [T, L], F32R, isOutput=False)
    xi_d = nc.declare_dram_parameter("xi", [T, L], F32R, isOutput=False)
    ar_d = nc.declare_dram_parameter("ar", [8 * P, P], F32R, isOutput=False)
    ai_d = nc.declare_dram_parameter("ai", [8 * P, P], F32R, isOutput=False)
    nai_d = nc.declare_dram_parameter("nai", [8 * P, P], F32R, isOutput=False)
    br_d = nc.declare_dram_parameter("br", [8 * P, P], BF16, isOutput=False)
    bi_d = nc.declare_dram_parameter("bi", [8 * P, P], BF16, isOutput=False)
    nbi_d = nc.declare_dram_parameter("nbi", [8 * P, P], BF16, isOutput=False)
    out_d = nc.declare_dram_parameter("out", [T, 2 * L], F32, isOutput=True)

    with tile.TileContext(nc) as tc:
        with (
            tc.tile_pool(name="const", bufs=1) as const_pool,
            tc.tile_pool(name="mats", bufs=1) as mat_pool,
            tc.tile_pool(name="x", bufs=8) as x_pool,
            tc.tile_pool(name="xt", bufs=20) as xt_pool,
            tc.tile_pool(name="ya", bufs=12) as ya_pool,
            tc.tile_pool(name="bin", bufs=1) as bin_pool,
            tc.tile_pool(name="yb", bufs=4) as yb_pool,
            tc.tile_pool(name="osb", bufs=4) as o_pool,
            tc.tile_pool(name="ps", bufs=8, space=bass.MemorySpace.PSUM) as ps_pool,
        ):
            ident = const_pool.tile([P, P], F32)
            make_identity(nc, ident[:])
            ident_h = const_pool.tile([P, P], BF16)
            nc.vector.tensor_copy(ident_h[:], ident[:])
            ident_r = const_pool.tile([P, P], F32R)
            nc.vector.tensor_copy(ident_r[:], ident[:])

            # HAM warmup while the (small) stationaries stream in.
            warm = ps_pool.tile([P, 4 * P], F32, tag="ps")
            for _ in range(22):
                for j in range(4):
                    nc.tensor.transpose(
                        warm[:, j * P : (j + 1) * P], ident[:], ident[:]
                    )

            # Persistent double-buffered shuffle destination; memset once so
            # downstream readers of the stepped-partition DMA writes are
            # observable (sim init tracking) — overlaps with warmup/mats DMA.
            bn_bufs = []
            bn_memsets = []
            for i in range(2):
                bnb = bin_pool.tile([P, 8, 2 * TC], BF16, tag=f"bin{i}")
                bn_memsets.append(nc.gpsimd.memset(bnb[:], 0.0))
                bn_bufs.append(bnb)

            # Mats go through the gpsimd SWDGE queues so the 48 dma_starts do
            # not serialize ahead of chunk-0 row loads on the two HWDGE queues.
            mats = {}
            for nm, dram, dt_ in (
                ("ar", ar_d, F32R), ("ai", ai_d, F32R), ("nai", nai_d, F32R),
                ("br", br_d, BF16), ("bi", bi_d, BF16), ("nbi", nbi_d, BF16),
            ):
                for r in range(8):
                    t_ = mat_pool.tile([P, P], dt_, tag=f"{nm}{r}")
                    nc.gpsimd.dma_start(out=t_[:], in_=dram[r * P : (r + 1) * P, :])
                    mats[nm, r] = t_

            def emit_front(ch):
                """T_in + stage A + shuffle for chunk ch."""
                tok0 = ch * TC
                rows = {}
                for pl, dram in ((0, xr_d), (1, xi_d)):
                    for tt in range(TC // P):
                        rt = x_pool.tile([P, P, 8], F32R, tag="rows")
                        r0 = tok0 + tt * P
                        eng = nc.scalar if (tt % 2) else nc.sync
                        eng.dma_start(out=rt[:], in_=dram[r0 : r0 + P, :])
                        rows[pl, tt] = rt

                xT = {}
                for pl in range(2):
                    for r in range(8):
                        tp = ps_pool.tile([P, 4 * P], F32R, tag="ps")
                        for tt in range(TC // P):
                            nc.tensor.transpose(
                                tp[:, tt * P : (tt + 1) * P],
                                rows[pl, tt][:, :, r],
                                ident_r[:],
                            )
                        dst = xt_pool.tile([P, TC], F32R, tag="xT")
                        nc.scalar.copy(dst[:], tp[:])
                        xT[pl, r] = dst

                yA = {}
                for r in range(8):
                    g = _rev(r, 3)
                    acr = ps_pool.tile([P, TC], F32, tag="ps")
                    aci = ps_pool.tile([P, TC], F32, tag="ps")
                    nc.tensor.matmul(acr[:], mats["ar", r][:], xT[0, r][:], start=True, stop=False)
                    nc.tensor.matmul(acr[:], mats["nai", r][:], xT[1, r][:], start=False, stop=True)
                    nc.tensor.matmul(aci[:], mats["ai", r][:], xT[0, r][:], start=True, stop=False)
                    nc.tensor.matmul(aci[:], mats["ar", r][:], xT[1, r][:], start=False, stop=True)
                    ya = ya_pool.tile([P, 2 * TC], BF16, tag="ya")
                    nc.vector.tensor_copy(ya[:, 0:TC], acr[:])
                    nc.vector.tensor_copy(ya[:, TC : 2 * TC], aci[:])
                    yA[g] = ya

                # shuffle: Bin[s*8+g, t2, :] = yA[g][s*8+t2, :] — one plain DMA
                # per g; one partition per SBUF port group on both sides.
                bn = bn_bufs[ch % 2]
                for g in range(8):
                    eng = nc.scalar if (g % 2) else nc.sync
                    eng.dma_start(out=bn[g:P:8, :, :], in_=yA[g][:])
                return bn

            def emit_back(ch, bn):
                """Stage B + T_out + interleave + store for chunk ch."""
                tok0 = ch * TC
                out_sb = []
                for tt in range(TC // P):
                    osb = o_pool.tile([P, 2 * L], F32, tag="osb")
                    out_sb.append(osb)
                for t2 in range(8):
                    obr = ps_pool.tile([P, TC], F32, tag="ps")
                    obi = ps_pool.tile([P, TC], F32, tag="ps")
                    b_re = bn[:, t2, 0:TC]
                    b_im = bn[:, t2, TC : 2 * TC]
                    nc.tensor.matmul(obr[:], mats["br", t2][:], b_re, start=True, stop=False)
                    nc.tensor.matmul(obr[:], mats["nbi", t2][:], b_im, start=False, stop=True)
                    nc.tensor.matmul(obi[:], mats["bi", t2][:], b_re, start=True, stop=False)
                    nc.tensor.matmul(obi[:], mats["br", t2][:], b_im, start=False, stop=True)
                    yb = yb_pool.tile([P, 2 * TC], BF16, tag="yb")
                    nc.scalar.copy(yb[:, 0:TC], obr[:])
                    nc.scalar.copy(yb[:, TC:], obi[:])

                    base = 2 * _rev(t2, 3)
                    for tt in range(TC // P):
                        tp2 = ps_pool.tile([P, 2, 16, 8], BF16, tag="ps")
                        nc.tensor.transpose(
                            tp2[:, 0], yb[:, tt * P : (tt + 1) * P], ident_h[:]
                        )
                        nc.tensor.transpose(
                            tp2[:, 1], yb[:, TC + tt * P : TC + (tt + 1) * P], ident_h[:]
                        )
                        osr = out_sb[tt][:].rearrange(
                            "q (m v lo) -> q lo v m", m=8, v=16, lo=16
                        )
                        nc.vector.tensor_copy(osr[:, base : base + 2, :, :], tp2[:])

                for tt in range(TC // P):
                    r0 = tok0 + tt * P
                    eng = nc.scalar if (tt % 2) else nc.sync
                    eng.dma_start(out=out_d[r0 : r0 + P, :], in_=out_sb[tt][:])

            # Software pipeline: back-half of chunk ch-1 is emitted after the
            # front-half (and shuffle issue) of chunk ch, so the PE stream has
            # B/T_out work in hand while chunk ch's shuffle is in flight.
            prev = None
            for ch in range(NCH):
                bn = emit_front(ch)
                if prev is not None:
                    emit_back(prev[0], prev[1])
                prev = (ch, bn)
            emit_back(prev[0], prev[1])

    nc.compile()
    return nc


def _build_program_v4():
    """Two-stage butterfly with all transposes moved to the host.

    x arrives pre-transposed and r-grouped in HBM as bf16 rows
    (plane, r, p) x tok, so stage-A moving operands are plain contiguous
    loads.  Stage A: acc[row', tok] = A_r^T x_r with the A/B stage
    matrices stationary; the stepped-partition SBUF shuffle regroups
    (s,t2) -> (s,g) partitions for stage B; stage-B results [j', tok]
    are stored position-major and the host undoes the butterfly output
    permutation + transpose.  No PE transposes, no output interleave.
    """
    nc = bacc.Bacc(
        "TRN2", target_bir_lowering=False, debug=False, num_devices=N_CORES,
        detect_race_conditions=False,
    )

    xt_d = nc.declare_dram_parameter("xt", [16 * P, T], BF16, isOutput=False)
    ar_d = nc.declare_dram_parameter("ar", [8 * P, P], BF16, isOutput=False)
    ai_d = nc.declare_dram_parameter("ai", [8 * P, P], BF16, isOutput=False)
    nai_d = nc.declare_dram_parameter("nai", [8 * P, P], BF16, isOutput=False)
    br_d = nc.declare_dram_parameter("br", [8 * P, P], BF16, isOutput=False)
    bi_d = nc.declare_dram_parameter("bi", [8 * P, P], BF16, isOutput=False)
    nbi_d = nc.declare_dram_parameter("nbi", [8 * P, P], BF16, isOutput=False)
    out_d = nc.declare_dram_parameter("out", [16 * P, T], F32, isOutput=True)

    with tile.TileContext(nc) as tc:
        with (
            tc.tile_pool(name="const", bufs=1) as const_pool,
            tc.tile_pool(name="mats", bufs=1) as mat_pool,
            tc.tile_pool(name="x", bufs=2) as x_pool,
            tc.tile_pool(name="ya", bufs=2) as ya_pool,
            tc.tile_pool(name="bin", bufs=1) as bin_pool,
            tc.tile_pool(name="osb", bufs=2) as o_pool,
            tc.tile_pool(name="ps", bufs=8, space=bass.MemorySpace.PSUM) as ps_pool,
        ):
            ident = const_pool.tile([P, P], F32)
            make_identity(nc, ident[:])

            # HAM warmup while the stationaries stream in.
            warm = ps_pool.tile([P, 4 * P], F32, tag="ps")
            for _ in range(22):
                for j in range(4):
                    nc.tensor.transpose(
                        warm[:, j * P : (j + 1) * P], ident[:], ident[:]
                    )

            # Persistent double-buffered shuffle destination (memset once for
            # sim init tracking of the stepped-partition DMA writes).
            bn_bufs = []
            for i in range(2):
                bnb = bin_pool.tile([P, 8, 2, TC], BF16, tag=f"bin{i}")
                nc.gpsimd.memset(bnb[:], 0.0)
                bn_bufs.append(bnb)

            # Stationaries through the gpsimd SWDGE queues so they don't
            # serialize ahead of chunk-0 x loads on the HWDGE queues.
            mats = {}
            for nm, dram in (
                ("ar", ar_d), ("ai", ai_d), ("nai", nai_d),
                ("br", br_d), ("bi", bi_d), ("nbi", nbi_d),
            ):
                for r in range(8):
                    t_ = mat_pool.tile([P, P], BF16, tag=f"{nm}{r}")
                    nc.gpsimd.dma_start(out=t_[:], in_=dram[r * P : (r + 1) * P, :])
                    mats[nm, r] = t_

            def emit_front(ch):
                """x loads + stage A + cast + shuffle for chunk ch."""
                csl = slice(ch * TC, (ch + 1) * TC)
                xs = {}
                for pl in range(2):
                    for r in range(8):
                        xtile = x_pool.tile([P, TC], BF16, tag=f"x{pl}_{r}")
                        row0 = (pl * 8 + r) * P
                        eng = nc.scalar if ((pl * 8 + r) % 2) else nc.sync
                        eng.dma_start(out=xtile[:], in_=xt_d[row0 : row0 + P, csl])
                        xs[pl, r] = xtile

                bn = bn_bufs[ch % 2]
                for r in range(8):
                    g = _rev(r, 3)
                    acr = ps_pool.tile([P, TC], F32, tag="ps")
                    aci = ps_pool.tile([P, TC], F32, tag="ps")
                    nc.tensor.matmul(acr[:], mats["ar", r][:], xs[0, r][:], start=True, stop=False)
                    nc.tensor.matmul(aci[:], mats["ar", r][:], xs[1, r][:], start=True, stop=False)
                    nc.tensor.matmul(aci[:], mats["ai", r][:], xs[0, r][:], start=False, stop=True)
                    nc.tensor.matmul(acr[:], mats["nai", r][:], xs[1, r][:], start=False, stop=True)
                    ya = ya_pool.tile([P, 2, TC], BF16, tag=f"ya{g}")
                    if r % 2:
                        nc.scalar.copy(ya[:, 0, :], acr[:])
                        nc.vector.tensor_copy(ya[:, 1, :], aci[:])
                    else:
                        nc.vector.tensor_copy(ya[:, 0, :], acr[:])
                        nc.scalar.copy(ya[:, 1, :], aci[:])
                    # Shuffle: bn[s*8+g, t2, c, :] = ya[s*8+t2, c, :]
                    eng = nc.scalar if (g % 2) else nc.sync
                    eng.dma_start(out=bn[g:P:8, :, :, :], in_=ya[:])
                return bn

            def emit_back(ch, bn):
                """Stage B + store (position-major) for chunk ch."""
                csl = slice(ch * TC, (ch + 1) * TC)
                for t2 in range(8):
                    obr = ps_pool.tile([P, TC], F32, tag="ps")
                    obi = ps_pool.tile([P, TC], F32, tag="ps")
                    b_re = bn[:, t2, 0, :]
                    b_im = bn[:, t2, 1, :]
                    nc.tensor.matmul(obr[:], mats["br", t2][:], b_re, start=True, stop=False)
                    nc.tensor.matmul(obi[:], mats["br", t2][:], b_im, start=True, stop=False)
                    nc.tensor.matmul(obi[:], mats["bi", t2][:], b_re, start=False, stop=True)
                    nc.tensor.matmul(obr[:], mats["nbi", t2][:], b_im, start=False, stop=True)
                    osr = o_pool.tile([P, TC], F32, tag=f"osr{t2}")
                    osi = o_pool.tile([P, TC], F32, tag=f"osi{t2}")
                    if t2 % 2:
                        nc.scalar.copy(osr[:], obr[:])
                        nc.vector.tensor_copy(osi[:], obi[:])
                    else:
                        nc.vector.tensor_copy(osr[:], obr[:])
                        nc.scalar.copy(osi[:], obi[:])
                    row_r = (t2 * 2) * P
                    row_i = (t2 * 2 + 1) * P
                    eng = nc.scalar if (t2 % 2) else nc.sync
                    eng2 = nc.sync if (t2 % 2) else nc.scalar
                    eng.dma_start(out=out_d[row_r : row_r + P, csl], in_=osr[:])
                    eng2.dma_start(out=out_d[row_i : row_i + P, csl], in_=osi[:])

            prev = None
            for ch in range(NCH):
                bn = emit_front(ch)
                if prev is not None:
                    emit_back(prev[0], prev[1])
                prev = (ch, bn)
            emit_back(prev[0], prev[1])

    nc.compile()
    return nc


_CACHED = {}


def _host_prep_v4(x_re, x_im, phases):
    """Host-side: transposed/r-grouped bf16 x per (core-half), bf16 mats."""
    import ml_dtypes

    Astat, Bstat = _stage_matrices(phases)
    bf = ml_dtypes.bfloat16
    ar = Astat.real.reshape(MESH_BATCH, 8 * P, P).astype(bf)
    ai = Astat.imag.reshape(MESH_BATCH, 8 * P, P).astype(bf)
    br = Bstat.real.reshape(MESH_BATCH, 8 * P, P).astype(bf)
    bi = Bstat.imag.reshape(MESH_BATCH, 8 * P, P).astype(bf)

    half = N_TOKENS // 2
    xts = []
    for h in range(2):
        planes = []
        for xp in (x_re, x_im):
            # [T, L] -> [L, T] -> (p, r) rows -> [r, p, T]
            xT = xp[h * half : (h + 1) * half].T.reshape(P, 8, half)
            planes.append(xT.transpose(1, 0, 2))
        xt = np.concatenate(planes, axis=0).reshape(16 * P, half)
        xts.append(np.ascontiguousarray(xt).astype(bf))
    return ar, ai, br, bi, xts


_JCOLS = None


def _jcols():
    global _JCOLS
    if _JCOLS is None:
        idx = np.arange(P)
        v_, m_ = np.divmod(idx, 8)
        _JCOLS = [P * m_ + 8 * v_ + _rev(t2, 3) for t2 in range(8)]
    return _JCOLS


def kernel(x_re: np.ndarray, x_im: np.ndarray, phases: np.ndarray) -> np.ndarray:
    global LAST_RESULTS

    x_re = np.ascontiguousarray(x_re, dtype=np.float32)
    x_im = np.ascontiguousarray(x_im, dtype=np.float32)
    phases = np.ascontiguousarray(phases, dtype=np.float32)

    half = N_TOKENS // 2
    in_maps = []
    if VERSION == 4:
        ar, ai, br, bi, xts = _host_prep_v4(x_re, x_im, phases)
        if 4 not in _CACHED:
            _CACHED[4] = _build_program_v4()
        nc = _CACHED[4]
        for c in range(N_CORES):
            b, h = c // 2, c % 2
            in_maps.append(
                {
                    "xt": xts[h],
                    "ar": ar[b],
                    "ai": ai[b],
                    "nai": np.ascontiguousarray(-ai[b]),
                    "br": br[b],
                    "bi": bi[b],
                    "nbi": np.ascontiguousarray(-bi[b]),
                }
            )
        res = run_bass_kernel_spmd(nc, in_maps, list(range(N_CORES)), trace=TRACE)
        LAST_RESULTS = res
        jcols = _jcols()
        out = np.empty((MESH_BATCH, N_TOKENS, L), dtype=np.complex64)
        for c in range(N_CORES):
            b, h = c // 2, c % 2
            arr = res.results[c]["out"].reshape(8, 2, P, half)
            sl = slice(h * half, (h + 1) * half)
            for t2 in range(8):
                cplx = (arr[t2, 0] + 1j * arr[t2, 1]).astype(np.complex64)
                out[b, sl, jcols[t2]] = cplx
        return out
    if VERSION == 2:
        W = _build_W(phases)                  # (B, L, L) complex64
        Wr = np.ascontiguousarray(W.real, dtype=np.float32)
        Wi = np.ascontiguousarray(W.imag, dtype=np.float32)
        if 2 not in _CACHED:
            _CACHED[2] = _build_program()
        nc = _CACHED[2]
        for c in range(N_CORES):
            b, h = c // 2, c % 2
            in_maps.append(
                {
                    "xr": x_re[h * half : (h + 1) * half],
                    "xi": x_im[h * half : (h + 1) * half],
                    "wr": Wr[b],
                    "wi": Wi[b],
                }
            )
    else:
        import ml_dtypes

        Astat, Bstat = _stage_matrices(phases)
        ar = np.ascontiguousarray(Astat.real.reshape(MESH_BATCH, 8 * P, P))
        ai = np.ascontiguousarray(Astat.imag.reshape(MESH_BATCH, 8 * P, P))
        br = Bstat.real.reshape(MESH_BATCH, 8 * P, P).astype(ml_dtypes.bfloat16)
        bi = Bstat.imag.reshape(MESH_BATCH, 8 * P, P).astype(ml_dtypes.bfloat16)
        if 3 not in _CACHED:
            _CACHED[3] = _build_program_v3()
        nc = _CACHED[3]
        for c in range(N_CORES):
            b, h = c // 2, c % 2
            in_maps.append(
                {
                    "xr": x_re[h * half : (h + 1) * half],
                    "xi": x_im[h * half : (h + 1) * half],
                    "ar": ar[b],
                    "ai": ai[b],
                    "nai": np.ascontiguousarray(-ai[b]),
                    "br": br[b],
                    "bi": bi[b],
                    "nbi": np.ascontiguousarray(-bi[b]),
                }
            )

    res = run_bass_kernel_spmd(nc, in_maps, list(range(N_CORES)), trace=TRACE)
    LAST_RESULTS = res

    out = np.empty((MESH_BATCH, N_TOKENS, L), dtype=np.complex64)
    for c in range(N_CORES):
        b, h = c // 2, c % 2
        out[b, h * half : (h + 1) * half] = (
            res.results[c]["out"].view(np.complex64).reshape(half, L)
        )
    return out



# revision 8
# speedup vs baseline: 1.4774x; 1.0599x over previous
"""Trainium2 Bass kernel for nn_BatchTrainableButterfly.

The reference applies, per mesh-batch b, a trainable butterfly network
(10 levels of phase shifters + 2x2 directional couplers with butterfly
permutations, plus a final phase layer and bit-reversals) to every token
row x[n, :].  For fixed phases the whole network is a linear map on
C^1024, so out[b] = x @ W_b with W_b = network_b(I_1024) — a 1024x1024
complex64 matrix that is cheap to build on host (O(L^2 log L) total).

Device work per core (8 cores = 4 mesh-batches x 2 token halves):
  out_half[b] = x_half @ W_b as real fp32r matmuls on TensorE:
    re = xr@Wr + xi@(-Wi),  im = xr@Wi + xi@Wr
x arrives token-major, so each 128-token tile is transposed on the PE
(L on partitions) to serve as the matmul stationary operand; results
accumulate in PSUM, are interleaved re/im into SBUF and DMA'd out as
complex64-compatible rows.
"""

import math

import numpy as np

import concourse.tile as tile
from concourse import bacc, bass, mybir
from concourse.bass_utils import run_bass_kernel_spmd
from concourse.masks import make_identity

P = 128          # partitions
L = 1024         # butterfly length
N_TOKENS = 4096
MESH_BATCH = 4
N_CORES = 8
T = (N_TOKENS * MESH_BATCH) // N_CORES  # 2048 token-rows per core
NT = T // P      # 16 token tiles per core
KC = L // P      # 8 contraction chunks
NLEV = int(math.log2(L))  # 10

F32 = mybir.dt.float32
F32R = mybir.dt.float32r
BF16 = mybir.dt.bfloat16

TC = 512          # tokens per pipeline chunk (v3)
NCH = T // TC     # 4 chunks

TRACE = False
LAST_RESULTS = None
VERSION = 4       # 2 = full-W matmul, 3 = two-stage, 4 = host-transposed two-stage

# ----------------------------------------------------------------------
# Host side: build the per-batch transfer matrices from the phases.
# ----------------------------------------------------------------------


def _bitrev(n):
    m = int(math.log2(n))
    perm = np.arange(n).reshape(n, 1)
    for _ in range(m):
        n1 = perm.shape[0] // 2
        perm = np.hstack((perm[:n1], perm[n1:]))
    return perm.squeeze(0)


def _forward_indices(length):
    idx = []
    ar = np.arange(length)
    for level in range(int(math.log2(length)) - 1):
        bs = 2 ** (level + 2)
        ind = ar.reshape(-1, length // bs, 2, bs // 2).transpose(0, 1, 3, 2)
        idx.append(ind.reshape(-1))
    return idx


def _build_W(phases):
    """phases (B, NLEV+1, L//2, 2) -> W (B, L, L) complex64 with out = x @ W."""
    B = phases.shape[0]
    br = _bitrev(L)
    fidx = _forward_indices(L)
    dc = np.array([[1.0, 1.0j], [1.0j, 1.0]], dtype=np.complex64)

    x = np.broadcast_to(np.eye(L, dtype=np.complex64), (B, L, L)).copy()
    x = x[..., br]
    for level in range(NLEV):
        x = x.reshape(B, L, L // 2, 2)
        ph = phases[:, level : level + 1, :, :]            # (B, 1, L//2, 2)
        x = x * np.exp(1j * ph.astype(np.complex64))
        x = x @ dc
        x = x.reshape(B, L, L)
        if level < NLEV - 1:
            x = x[..., fidx[level]]
    ph = phases[:, NLEV - 1 : NLEV, :, :].reshape(B, 1, L)
    x = x * np.exp(1j * ph.astype(np.complex64))
    x = x[..., br]
    return (x / np.float32(np.sqrt(L))).astype(np.complex64)


def _rev(v, n):
    r = 0
    for _ in range(n):
        r = (r << 1) | (v & 1)
        v >>= 1
    return r


def _stage_matrices(phases):
    """Two-stage factorization of the butterfly network.

    Stage A = input bitrev + levels 0..6 (perms 0..5, no trailing perm):
    block-diagonal; column-block g is fed by x columns {i : i = 8p + r},
    r = rev3(g).  Stage B = perm fidx[6] + levels 7..9 + final phase +
    final bitrev + scale: per-position 8x8 mixing across the 8 blocks.

    Returns per batch the PE stationaries:
      Astat[b, r] (128,128) cplx : lhsT with K=p (x idx 8p+r), M=pos.
      Bstat[b,t2] (128,128) cplx : lhsT with K = g*16+s (source y(g, t2*16+s)),
                                   M = v*8+m -> out col j = 128m + 8v + rev3(t2).
    Cross-component entries of the extracted B submatrix are exactly 0.
    """
    B_ = phases.shape[0]
    br = _bitrev(L)
    fidx = _forward_indices(L)
    dc = np.array([[1.0, 1.0j], [1.0j, 1.0]], dtype=np.complex64)

    def levels(x, lo, hi, pre_br=False, post_final=False, pre_perm=None):
        if pre_br:
            x = x[..., br]
        if pre_perm is not None:
            x = x[..., pre_perm]
        for level in range(lo, hi):
            x = x.reshape(B_, L, L // 2, 2)
            x = x * np.exp(1j * phases[:, level, None, :, :].astype(np.complex64))
            x = x @ dc
            x = x.reshape(B_, L, L)
            if level < NLEV - 1 and level != 6:
                x = x[..., fidx[level]]
        if post_final:
            x = x * np.exp(
                1j * phases[:, NLEV - 1, None, :, :].reshape(B_, 1, L).astype(np.complex64)
            )
            x = x[..., br]
            x = x / np.float32(np.sqrt(L))
        return x

    eye = np.broadcast_to(np.eye(L, dtype=np.complex64), (B_, L, L)).copy()
    A = levels(eye.copy(), 0, 7, pre_br=True)
    Bm = levels(eye.copy(), 7, NLEV, post_final=True, pre_perm=fidx[6])

    # Stage-A output row order: row' = s*8 + t2 for pos p'' = t2*16 + s, so the
    # inter-stage shuffle is one plain DMA per g: yA_g[:] -> Bin[g:128:8,:,:]
    # (dst partition k = s*8 + g, free = (t2, tok)).
    ar_ = np.arange(P)
    posperm = (ar_ & 7) * 16 + (ar_ >> 3)          # row' -> p''
    Astat = np.empty((B_, 8, P, P), dtype=np.complex64)
    for r in range(8):
        g = _rev(r, 3)
        Astat[:, r] = A[:, ar_ * 8 + r][:, :, g * P + posperm]

    s_, g_ = np.divmod(ar_, 8)                     # k = s*8 + g
    v_, m_ = np.divmod(ar_, 8)
    Bstat = np.empty((B_, 8, P, P), dtype=np.complex64)
    for t2 in range(8):
        rows = g_ * P + t2 * 16 + s_
        cols = P * m_ + 8 * v_ + _rev(t2, 3)
        Bstat[:, t2] = Bm[:, rows][:, :, cols]
    return Astat, Bstat


# ----------------------------------------------------------------------
# Device side: complex matmul kernel (SPMD, one (batch, half) per core).
# ----------------------------------------------------------------------

_CACHED_NC = None


def _build_program():
    nc = bacc.Bacc(
        "TRN2", target_bir_lowering=False, debug=False, num_devices=N_CORES
    )

    xr_d = nc.declare_dram_parameter("xr", [T, L], F32, isOutput=False)
    xi_d = nc.declare_dram_parameter("xi", [T, L], F32, isOutput=False)
    wr_d = nc.declare_dram_parameter("wr", [L, L], F32R, isOutput=False)
    wi_d = nc.declare_dram_parameter("wi", [L, L], F32R, isOutput=False)
    out_d = nc.declare_dram_parameter("out", [T, 2 * L], F32, isOutput=True)

    with tile.TileContext(nc) as tc:
        with (
            tc.tile_pool(name="const", bufs=1) as const_pool,
            tc.tile_pool(name="w", bufs=1) as w_pool,
            tc.tile_pool(name="x", bufs=3) as x_pool,
            tc.tile_pool(name="xt", bufs=2) as xt_pool,
            tc.tile_pool(name="osb", bufs=3) as o_pool,
            tc.tile_pool(name="ps", bufs=8, space=bass.MemorySpace.PSUM) as ps_pool,
        ):
            ident = const_pool.tile([P, P], F32)
            make_identity(nc, ident[:])

            # Warm the PE HAM while W streams in: dummy transposes keep the
            # tensor engine busy >3.4us so it reaches full clock before the
            # real matmuls start.
            warm = ps_pool.tile([P, 4 * P], F32, tag="ps")
            for _ in range(12):
                for j in range(4):
                    nc.tensor.transpose(
                        warm[:, j * P : (j + 1) * P], ident[:], ident[:]
                    )

            # Stream W into SBUF once: per k-chunk tiles (P x L), natural layout
            # (partition = contraction row within chunk, free = output column).
            # k-major order so the first token tile's accumulation can start
            # after only a few chunks have landed.
            w_sb = {}
            for k in range(KC):
                for nm, dram in (("wr", wr_d), ("wi", wi_d)):
                    t_ = w_pool.tile([P, L], F32R, tag=f"{nm}{k}")
                    nc.sync.dma_start(out=t_[:], in_=dram[k * P : (k + 1) * P, :])
                    w_sb[nm, k] = t_
                # -Wi derived on device: saves a third of the W stream, which
                # gates the kernel head while PE waits on weights.
                nwi = w_pool.tile([P, L], F32R, tag=f"nwi{k}")
                nc.vector.tensor_scalar_mul(nwi[:], w_sb["wi", k][:], -1.0)
                w_sb["nwi", k] = nwi

            for t in range(NT):
                rows = slice(t * P, (t + 1) * P)
                xr_rows = x_pool.tile([P, L], F32, tag="xr_rows")
                xi_rows = x_pool.tile([P, L], F32, tag="xi_rows")
                nc.sync.dma_start(out=xr_rows[:], in_=xr_d[rows, :])
                nc.sync.dma_start(out=xi_rows[:], in_=xi_d[rows, :])

                # Transpose the token tile: xT chunks live at
                # xT[:, k*P:(k+1)*P] = x_rows[:, k*P:(k+1)*P].T
                xrT = xt_pool.tile([P, L], F32R, tag="xrT")
                xiT = xt_pool.tile([P, L], F32R, tag="xiT")
                for src, dst in ((xr_rows, xrT), (xi_rows, xiT)):
                    for g in range(2):
                        tp = ps_pool.tile([P, 4 * P], F32, tag="ps")
                        for j in range(4):
                            k = g * 4 + j
                            nc.tensor.transpose(
                                tp[:, j * P : (j + 1) * P],
                                src[:, k * P : (k + 1) * P],
                                ident[:],
                            )
                        nc.scalar.copy(dst[:, g * 4 * P : (g + 1) * 4 * P], tp[:])

                # Accumulate the four real matmul outputs.
                #   re_n = sum_k xrT_k @ wr_k[n] + xiT_k @ nwi_k[n]
                #   im_n = sum_k xrT_k @ wi_k[n] + xiT_k @ wr_k[n]
                out_sb = o_pool.tile([P, L, 2], F32, tag="out_sb")
                for n in range(2):
                    ncol = slice(n * 512, (n + 1) * 512)
                    acc_re = ps_pool.tile([P, 512], F32, tag="ps")
                    acc_im = ps_pool.tile([P, 512], F32, tag="ps")
                    for k in range(KC):
                        xrT_k = xrT[:, k * P : (k + 1) * P]
                        xiT_k = xiT[:, k * P : (k + 1) * P]
                        first = k == 0
                        last = k == KC - 1
                        nc.tensor.matmul(
                            acc_re[:], xrT_k, w_sb["wr", k][:, ncol],
                            start=first, stop=False,
                        )
                        nc.tensor.matmul(
                            acc_re[:], xiT_k, w_sb["nwi", k][:, ncol],
                            start=False, stop=last,
                        )
                        nc.tensor.matmul(
                            acc_im[:], xrT_k, w_sb["wi", k][:, ncol],
                            start=first, stop=False,
                        )
                        nc.tensor.matmul(
                            acc_im[:], xiT_k, w_sb["wr", k][:, ncol],
                            start=False, stop=last,
                        )
                    # Interleave re/im into complex64 memory order.
                    nc.vector.tensor_copy(out_sb[:, n * 512 : (n + 1) * 512, 0], acc_re[:])
                    nc.vector.tensor_copy(out_sb[:, n * 512 : (n + 1) * 512, 1], acc_im[:])

                nc.sync.dma_start(out=out_d[rows, :], in_=out_sb[:])

    nc.compile()
    return nc


def _build_program_v3():
    # detect_race_conditions=False: the rust race detector false-positives on
    # the stepped-partition shuffle DMA vs writes to a *different* bin buffer
    # (disjoint SBUF regions sharing a shadow zone). Same-tensor deps are
    # tracked normally and validated by the CoreSim numeric check.
    nc = bacc.Bacc(
        "TRN2", target_bir_lowering=False, debug=False, num_devices=N_CORES,
        detect_race_conditions=False,
    )

    xr_d = nc.declare_dram_parameter("xr", [T, L], F32R, isOutput=False)
    xi_d = nc.declare_dram_parameter("xi", [T, L], F32R, isOutput=False)
    ar_d = nc.declare_dram_parameter("ar", [8 * P, P], F32R, isOutput=False)
    ai_d = nc.declare_dram_parameter("ai", [8 * P, P], F32R, isOutput=False)
    nai_d = nc.declare_dram_parameter("nai", [8 * P, P], F32R, isOutput=False)
    br_d = nc.declare_dram_parameter("br", [8 * P, P], BF16, isOutput=False)
    bi_d = nc.declare_dram_parameter("bi", [8 * P, P], BF16, isOutput=False)
    nbi_d = nc.declare_dram_parameter("nbi", [8 * P, P], BF16, isOutput=False)
    out_d = nc.declare_dram_parameter("out", [T, 2 * L], F32, isOutput=True)

    with tile.TileContext(nc) as tc:
        with (
            tc.tile_pool(name="const", bufs=1) as const_pool,
            tc.tile_pool(name="mats", bufs=1) as mat_pool,
            tc.tile_pool(name="x", bufs=8) as x_pool,
            tc.tile_pool(name="xt", bufs=20) as xt_pool,
            tc.tile_pool(name="ya", bufs=12) as ya_pool,
            tc.tile_pool(name="bin", bufs=1) as bin_pool,
            tc.tile_pool(name="yb", bufs=4) as yb_pool,
            tc.tile_pool(name="osb", bufs=4) as o_pool,
            tc.tile_pool(name="ps", bufs=8, space=bass.MemorySpace.PSUM) as ps_pool,
        ):
            ident = const_pool.tile([P, P], F32)
            make_identity(nc, ident[:])
            ident_h = const_pool.tile([P, P], BF16)
            nc.vector.tensor_copy(ident_h[:], ident[:])
            ident_r = const_pool.tile([P, P], F32R)
            nc.vector.tensor_copy(ident_r[:], ident[:])

            # HAM warmup while the (small) stationaries stream in.
            warm = ps_pool.tile([P, 4 * P], F32, tag="ps")
            for _ in range(22):
                for j in range(4):
                    nc.tensor.transpose(
                        warm[:, j * P : (j + 1) * P], ident[:], ident[:]
                    )

            # Persistent double-buffered shuffle destination; memset once so
            # downstream readers of the stepped-partition DMA writes are
            # observable (sim init tracking) — overlaps with warmup/mats DMA.
            bn_bufs = []
            bn_memsets = []
            for i in range(2):
                bnb = bin_pool.tile([P, 8, 2 * TC], BF16, tag=f"bin{i}")
                bn_memsets.append(nc.gpsimd.memset(bnb[:], 0.0))
                bn_bufs.append(bnb)

            # Mats go through the gpsimd SWDGE queues so the 48 dma_starts do
            # not serialize ahead of chunk-0 row loads on the two HWDGE queues.
            mats = {}
            for nm, dram, dt_ in (
                ("ar", ar_d, F32R), ("ai", ai_d, F32R), ("nai", nai_d, F32R),
                ("br", br_d, BF16), ("bi", bi_d, BF16), ("nbi", nbi_d, BF16),
            ):
                for r in range(8):
                    t_ = mat_pool.tile([P, P], dt_, tag=f"{nm}{r}")
                    nc.gpsimd.dma_start(out=t_[:], in_=dram[r * P : (r + 1) * P, :])
                    mats[nm, r] = t_

            def emit_front(ch):
                """T_in + stage A + shuffle for chunk ch."""
                tok0 = ch * TC
                rows = {}
                for pl, dram in ((0, xr_d), (1, xi_d)):
                    for tt in range(TC // P):
                        rt = x_pool.tile([P, P, 8], F32R, tag="rows")
                        r0 = tok0 + tt * P
                        eng = nc.scalar if (tt % 2) else nc.sync
                        eng.dma_start(out=rt[:], in_=dram[r0 : r0 + P, :])
                        rows[pl, tt] = rt

                xT = {}
                for pl in range(2):
                    for r in range(8):
                        tp = ps_pool.tile([P, 4 * P], F32R, tag="ps")
                        for tt in range(TC // P):
                            nc.tensor.transpose(
                                tp[:, tt * P : (tt + 1) * P],
                                rows[pl, tt][:, :, r],
                                ident_r[:],
                            )
                        dst = xt_pool.tile([P, TC], F32R, tag="xT")
                        nc.scalar.copy(dst[:], tp[:])
                        xT[pl, r] = dst

                yA = {}
                for r in range(8):
                    g = _rev(r, 3)
                    acr = ps_pool.tile([P, TC], F32, tag="ps")
                    aci = ps_pool.tile([P, TC], F32, tag="ps")
                    nc.tensor.matmul(acr[:], mats["ar", r][:], xT[0, r][:], start=True, stop=False)
                    nc.tensor.matmul(acr[:], mats["nai", r][:], xT[1, r][:], start=False, stop=True)
                    nc.tensor.matmul(aci[:], mats["ai", r][:], xT[0, r][:], start=True, stop=False)
                    nc.tensor.matmul(aci[:], mats["ar", r][:], xT[1, r][:], start=False, stop=True)
                    ya = ya_pool.tile([P, 2 * TC], BF16, tag="ya")
                    nc.vector.tensor_copy(ya[:, 0:TC], acr[:])
                    nc.vector.tensor_copy(ya[:, TC : 2 * TC], aci[:])
                    yA[g] = ya

                # shuffle: Bin[s*8+g, t2, :] = yA[g][s*8+t2, :] — one plain DMA
                # per g; one partition per SBUF port group on both sides.
                bn = bn_bufs[ch % 2]
                for g in range(8):
                    eng = nc.scalar if (g % 2) else nc.sync
                    eng.dma_start(out=bn[g:P:8, :, :], in_=yA[g][:])
                return bn

            def emit_back(ch, bn):
                """Stage B + T_out + interleave + store for chunk ch."""
                tok0 = ch * TC
                out_sb = []
                for tt in range(TC // P):
                    osb = o_pool.tile([P, 2 * L], F32, tag="osb")
                    out_sb.append(osb)
                for t2 in range(8):
                    obr = ps_pool.tile([P, TC], F32, tag="ps")
                    obi = ps_pool.tile([P, TC], F32, tag="ps")
                    b_re = bn[:, t2, 0:TC]
                    b_im = bn[:, t2, TC : 2 * TC]
                    nc.tensor.matmul(obr[:], mats["br", t2][:], b_re, start=True, stop=False)
                    nc.tensor.matmul(obr[:], mats["nbi", t2][:], b_im, start=False, stop=True)
                    nc.tensor.matmul(obi[:], mats["bi", t2][:], b_re, start=True, stop=False)
                    nc.tensor.matmul(obi[:], mats["br", t2][:], b_im, start=False, stop=True)
                    yb = yb_pool.tile([P, 2 * TC], BF16, tag="yb")
                    nc.scalar.copy(yb[:, 0:TC], obr[:])
                    nc.scalar.copy(yb[:, TC:], obi[:])

                    base = 2 * _rev(t2, 3)
                    for tt in range(TC // P):
                        tp2 = ps_pool.tile([P, 2, 16, 8], BF16, tag="ps")
                        nc.tensor.transpose(
                            tp2[:, 0], yb[:, tt * P : (tt + 1) * P], ident_h[:]
                        )
                        nc.tensor.transpose(
                            tp2[:, 1], yb[:, TC + tt * P : TC + (tt + 1) * P], ident_h[:]
                        )
                        osr = out_sb[tt][:].rearrange(
                            "q (m v lo) -> q lo v m", m=8, v=16, lo=16
                        )
                        nc.vector.tensor_copy(osr[:, base : base + 2, :, :], tp2[:])

                for tt in range(TC // P):
                    r0 = tok0 + tt * P
                    eng = nc.scalar if (tt % 2) else nc.sync
                    eng.dma_start(out=out_d[r0 : r0 + P, :], in_=out_sb[tt][:])

            # Software pipeline: back-half of chunk ch-1 is emitted after the
            # front-half (and shuffle issue) of chunk ch, so the PE stream has
            # B/T_out work in hand while chunk ch's shuffle is in flight.
            prev = None
            for ch in range(NCH):
                bn = emit_front(ch)
                if prev is not None:
                    emit_back(prev[0], prev[1])
                prev = (ch, bn)
            emit_back(prev[0], prev[1])

    nc.compile()
    return nc


def _build_program_v4():
    """Two-stage butterfly with all transposes moved to the host.

    x arrives pre-transposed and r-grouped in HBM as bf16 rows
    (plane, r, p) x tok, so stage-A moving operands are plain contiguous
    loads.  Stage A: acc[row', tok] = A_r^T x_r with the A/B stage
    matrices stationary; the stepped-partition SBUF shuffle regroups
    (s,t2) -> (s,g) partitions for stage B; stage-B results [j', tok]
    are stored position-major and the host undoes the butterfly output
    permutation + transpose.  No PE transposes, no output interleave.
    """
    nc = bacc.Bacc(
        "TRN2", target_bir_lowering=False, debug=False, num_devices=N_CORES,
        detect_race_conditions=False,
    )

    xt_d = nc.declare_dram_parameter("xt", [16 * P, T], BF16, isOutput=False)
    ar_d = nc.declare_dram_parameter("ar", [8 * P, P], BF16, isOutput=False)
    ai_d = nc.declare_dram_parameter("ai", [8 * P, P], BF16, isOutput=False)
    nai_d = nc.declare_dram_parameter("nai", [8 * P, P], BF16, isOutput=False)
    br_d = nc.declare_dram_parameter("br", [8 * P, P], BF16, isOutput=False)
    bi_d = nc.declare_dram_parameter("bi", [8 * P, P], BF16, isOutput=False)
    nbi_d = nc.declare_dram_parameter("nbi", [8 * P, P], BF16, isOutput=False)
    out_d = nc.declare_dram_parameter("out", [16 * P, T], F32, isOutput=True)

    with tile.TileContext(nc) as tc:
        with (
            tc.tile_pool(name="const", bufs=1) as const_pool,
            tc.tile_pool(name="mats", bufs=1) as mat_pool,
            tc.tile_pool(name="x", bufs=1) as x_pool,
            tc.tile_pool(name="ya", bufs=2) as ya_pool,
            tc.tile_pool(name="bin", bufs=1) as bin_pool,
            tc.tile_pool(name="osb", bufs=2) as o_pool,
            tc.tile_pool(name="ps", bufs=8, space=bass.MemorySpace.PSUM) as ps_pool,
        ):
            ident = const_pool.tile([P, P], F32)
            make_identity(nc, ident[:])

            # HAM warmup; starts as soon as ident lands.
            warm = ps_pool.tile([P, 4 * P], F32, tag="ps")
            for _ in range(22):
                for j in range(4):
                    nc.tensor.transpose(
                        warm[:, j * P : (j + 1) * P], ident[:], ident[:]
                    )

            # Persistent double-buffered shuffle destination (memset once for
            # sim init tracking of the stepped-partition DMA writes).  DVE
            # memsets finish in ~9us, well before the first shuffle.
            bn_bufs = []
            for i in range(2):
                bnb = bin_pool.tile([P, 8, 2, TC], BF16, tag=f"bin{i}")
                nc.vector.memset(bnb[:], 0.0)
                bn_bufs.append(bnb)

            # Stationaries first on the HWDGE queues (1.5 MB, ~4us), then the
            # full x panel as 16 big DMAs with 4 KiB partition lines.  r=0's
            # tiles land first so stage A can start immediately after.
            mats = {}
            for r in range(8):
                for qi, (nm, dram) in enumerate((
                    ("ar", ar_d), ("ai", ai_d), ("nai", nai_d),
                    ("br", br_d), ("bi", bi_d), ("nbi", nbi_d),
                )):
                    t_ = mat_pool.tile([P, P], BF16, tag=f"{nm}{r}")
                    eng = nc.scalar if (qi % 2) else nc.sync
                    eng.dma_start(out=t_[:], in_=dram[r * P : (r + 1) * P, :])
                    mats[nm, r] = t_

            xs = {}
            for r in range(8):
                for pl in range(2):
                    xtile = x_pool.tile([P, T], BF16, tag=f"x{pl}_{r}")
                    row0 = (pl * 8 + r) * P
                    eng = nc.scalar if (pl % 2) else nc.sync
                    eng.dma_start(out=xtile[:], in_=xt_d[row0 : row0 + P, :])
                    xs[pl, r] = xtile

            def emit_front(ch):
                """Stage A + cast + shuffle for chunk ch."""
                csl = slice(ch * TC, (ch + 1) * TC)
                bn = bn_bufs[ch % 2]
                for r in range(8):
                    g = _rev(r, 3)
                    acr = ps_pool.tile([P, TC], F32, tag="ps")
                    aci = ps_pool.tile([P, TC], F32, tag="ps")
                    nc.tensor.matmul(acr[:], mats["ar", r][:], xs[0, r][:, csl], start=True, stop=False)
                    nc.tensor.matmul(aci[:], mats["ar", r][:], xs[1, r][:, csl], start=True, stop=False)
                    nc.tensor.matmul(aci[:], mats["ai", r][:], xs[0, r][:, csl], start=False, stop=True)
                    nc.tensor.matmul(acr[:], mats["nai", r][:], xs[1, r][:, csl], start=False, stop=True)
                    ya = ya_pool.tile([P, 2, TC], BF16, tag=f"ya{g}")
                    if r % 2:
                        nc.scalar.copy(ya[:, 0, :], acr[:])
                        nc.vector.tensor_copy(ya[:, 1, :], aci[:])
                    else:
                        nc.vector.tensor_copy(ya[:, 0, :], acr[:])
                        nc.scalar.copy(ya[:, 1, :], aci[:])
                    # Shuffle: bn[s*8+g, t2, c, :] = ya[s*8+t2, c, :]
                    nc.sync.dma_start(out=bn[g:P:8, :, :, :], in_=ya[:])
                return bn

            def emit_back(ch, bn):
                """Stage B + store (position-major) for chunk ch."""
                csl = slice(ch * TC, (ch + 1) * TC)
                for t2 in range(8):
                    obr = ps_pool.tile([P, TC], F32, tag="ps")
                    obi = ps_pool.tile([P, TC], F32, tag="ps")
                    b_re = bn[:, t2, 0, :]
                    b_im = bn[:, t2, 1, :]
                    nc.tensor.matmul(obr[:], mats["br", t2][:], b_re, start=True, stop=False)
                    nc.tensor.matmul(obi[:], mats["br", t2][:], b_im, start=True, stop=False)
                    nc.tensor.matmul(obi[:], mats["bi", t2][:], b_re, start=False, stop=True)
                    nc.tensor.matmul(obr[:], mats["nbi", t2][:], b_im, start=False, stop=True)
                    osr = o_pool.tile([P, TC], F32, tag=f"osr{t2}")
                    osi = o_pool.tile([P, TC], F32, tag=f"osi{t2}")
                    if t2 % 2:
                        nc.scalar.copy(osr[:], obr[:])
                        nc.vector.tensor_copy(osi[:], obi[:])
                    else:
                        nc.vector.tensor_copy(osr[:], obr[:])
                        nc.scalar.copy(osi[:], obi[:])
                    row_r = (t2 * 2) * P
                    row_i = (t2 * 2 + 1) * P
                    nc.scalar.dma_start(out=out_d[row_r : row_r + P, csl], in_=osr[:])
                    nc.scalar.dma_start(out=out_d[row_i : row_i + P, csl], in_=osi[:])

            prev = None
            for ch in range(NCH):
                bn = emit_front(ch)
                if prev is not None:
                    emit_back(prev[0], prev[1])
                prev = (ch, bn)
            emit_back(prev[0], prev[1])

    nc.compile()
    return nc


_CACHED = {}


def _host_prep_v4(x_re, x_im, phases):
    """Host-side: transposed/r-grouped bf16 x per (core-half), bf16 mats."""
    import ml_dtypes

    Astat, Bstat = _stage_matrices(phases)
    bf = ml_dtypes.bfloat16
    ar = Astat.real.reshape(MESH_BATCH, 8 * P, P).astype(bf)
    ai = Astat.imag.reshape(MESH_BATCH, 8 * P, P).astype(bf)
    br = Bstat.real.reshape(MESH_BATCH, 8 * P, P).astype(bf)
    bi = Bstat.imag.reshape(MESH_BATCH, 8 * P, P).astype(bf)

    half = N_TOKENS // 2
    xts = []
    for h in range(2):
        planes = []
        for xp in (x_re, x_im):
            # [T, L] -> [L, T] -> (p, r) rows -> [r, p, T]
            xT = xp[h * half : (h + 1) * half].T.reshape(P, 8, half)
            planes.append(xT.transpose(1, 0, 2))
        xt = np.concatenate(planes, axis=0).reshape(16 * P, half)
        xts.append(np.ascontiguousarray(xt).astype(bf))
    return ar, ai, br, bi, xts


_JCOLS = None


def _jcols():
    global _JCOLS
    if _JCOLS is None:
        idx = np.arange(P)
        v_, m_ = np.divmod(idx, 8)
        _JCOLS = [P * m_ + 8 * v_ + _rev(t2, 3) for t2 in range(8)]
    return _JCOLS


def kernel(x_re: np.ndarray, x_im: np.ndarray, phases: np.ndarray) -> np.ndarray:
    global LAST_RESULTS

    x_re = np.ascontiguousarray(x_re, dtype=np.float32)
    x_im = np.ascontiguousarray(x_im, dtype=np.float32)
    phases = np.ascontiguousarray(phases, dtype=np.float32)

    half = N_TOKENS // 2
    in_maps = []
    if VERSION == 4:
        ar, ai, br, bi, xts = _host_prep_v4(x_re, x_im, phases)
        if 4 not in _CACHED:
            _CACHED[4] = _build_program_v4()
        nc = _CACHED[4]
        for c in range(N_CORES):
            b, h = c // 2, c % 2
            in_maps.append(
                {
                    "xt": xts[h],
                    "ar": ar[b],
                    "ai": ai[b],
                    "nai": np.ascontiguousarray(-ai[b]),
                    "br": br[b],
                    "bi": bi[b],
                    "nbi": np.ascontiguousarray(-bi[b]),
                }
            )
        res = run_bass_kernel_spmd(nc, in_maps, list(range(N_CORES)), trace=TRACE)
        LAST_RESULTS = res
        jcols = _jcols()
        out = np.empty((MESH_BATCH, N_TOKENS, L), dtype=np.complex64)
        for c in range(N_CORES):
            b, h = c // 2, c % 2
            arr = res.results[c]["out"].reshape(8, 2, P, half)
            sl = slice(h * half, (h + 1) * half)
            for t2 in range(8):
                cplx = (arr[t2, 0] + 1j * arr[t2, 1]).astype(np.complex64)
                out[b, sl, jcols[t2]] = cplx
        return out
    if VERSION == 2:
        W = _build_W(phases)                  # (B, L, L) complex64
        Wr = np.ascontiguousarray(W.real, dtype=np.float32)
        Wi = np.ascontiguousarray(W.imag, dtype=np.float32)
        if 2 not in _CACHED:
            _CACHED[2] = _build_program()
        nc = _CACHED[2]
        for c in range(N_CORES):
            b, h = c // 2, c % 2
            in_maps.append(
                {
                    "xr": x_re[h * half : (h + 1) * half],
                    "xi": x_im[h * half : (h + 1) * half],
                    "wr": Wr[b],
                    "wi": Wi[b],
                }
            )
    else:
        import ml_dtypes

        Astat, Bstat = _stage_matrices(phases)
        ar = np.ascontiguousarray(Astat.real.reshape(MESH_BATCH, 8 * P, P))
        ai = np.ascontiguousarray(Astat.imag.reshape(MESH_BATCH, 8 * P, P))
        br = Bstat.real.reshape(MESH_BATCH, 8 * P, P).astype(ml_dtypes.bfloat16)
        bi = Bstat.imag.reshape(MESH_BATCH, 8 * P, P).astype(ml_dtypes.bfloat16)
        if 3 not in _CACHED:
            _CACHED[3] = _build_program_v3()
        nc = _CACHED[3]
        for c in range(N_CORES):
            b, h = c // 2, c % 2
            in_maps.append(
                {
                    "xr": x_re[h * half : (h + 1) * half],
                    "xi": x_im[h * half : (h + 1) * half],
                    "ar": ar[b],
                    "ai": ai[b],
                    "nai": np.ascontiguousarray(-ai[b]),
                    "br": br[b],
                    "bi": bi[b],
                    "nbi": np.ascontiguousarray(-bi[b]),
                }
            )

    res = run_bass_kernel_spmd(nc, in_maps, list(range(N_CORES)), trace=TRACE)
    LAST_RESULTS = res

    out = np.empty((MESH_BATCH, N_TOKENS, L), dtype=np.complex64)
    for c in range(N_CORES):
        b, h = c // 2, c % 2
        out[b, h * half : (h + 1) * half] = (
            res.results[c]["out"].view(np.complex64).reshape(half, L)
        )
    return out

